# revision 17
# baseline (speedup 1.0000x reference)
"""Causal MHA (B=4, N=2048, F=1024, H=16, D=64) on 8 TRN2 NeuronCores.

Sharding: core c -> batch c//2, head-group c%2 (8 heads each). No
cross-core communication.

v4 = v2 head-pair packing + pipeline shift + projection interleave.
 - v2 structure kept: scores for head pair (2p, 2p+1) issue as adjacent
   row-group-packed matmuls into ONE [128, 2*CW] PSUM tile; one exp per
   kj covers both heads; PV packs the pair into po col groups.
 - PV/denom for kj are issued one iteration LATE (after scores/exp of
   kj+1), so the PE FIFO queue always has exp-independent work and
   never stalls on the scalar engine.
 - projection 8-matmul chains interleave between attention iterations
   (paced schedule), instead of running as serial blocks.
 - output stored (d, n)-major f32 straight from the PV accumulator (one
   copy + one plain DMA per chunk; NO DMA transposes); host fixes
   layout + normalizes.
 - denominators: full off-diagonal ex tiles chain-summed 4-deep on DVE
   before each ones-matmul (half the denominator matmuls of v2).
 - diagonal causal masks: both heads' 128-col strips in ONE strided
   tensor_mul against a doubled mask tile.
 - PE warm-up matmul chain + dummy exp (ACT table preload) overlap the
   initial DMAs; input DMAs are chunked so the first scores issue ~5us
   in (w row-blocks jt0 first, x column-chunk 0, wv, rest).
"""

import sys
import time

sys.path.insert(0, "/opt/trn_rl_repo")

import ml_dtypes
import numpy as np

import concourse.bacc as bacc
import concourse.mybir as mybir
import concourse.tile as tile
from concourse.bass_utils import run_bass_kernel_spmd

B, N, F, H = 4, 2048, 1024, 16
D = 64
NCORES = 8
HL = H // 2          # heads per core
NP = HL // 2         # head pairs per core (4)
GC = HL * D          # per-core projection width (512)
P = 128
FT = F // P          # 8 contraction tiles
JT = GC // P         # 4 row tiles of QT/KT (one per head pair)
ST = N // P          # 16 seq tiles
CW = 512             # query chunk width
QC = N // CW         # 4 query chunks
BF16 = mybir.dt.bfloat16
F32 = mybir.dt.float32
EXPF = mybir.ActivationFunctionType.Exp
WARM_MM = 64         # warm-up matmuls (~7us at cold clock)
DSUM_S = 4           # ex tiles chain-summed per denominator matmul

_NC_CACHE = None


def _build():
    t0 = time.time()
    print("building bass graph...", flush=True)
    nc = bacc.Bacc("TRN2", target_bir_lowering=False, debug=False,
                   num_devices=NCORES)
    xT_d = nc.dram_tensor("xT", [F, N], BF16, kind="ExternalInput")
    wq_d = nc.dram_tensor("wq", [F, GC], BF16, kind="ExternalInput")
    wk_d = nc.dram_tensor("wk", [F, GC], BF16, kind="ExternalInput")
    wv_d = nc.dram_tensor("wv", [F, GC], BF16, kind="ExternalInput")
    msk_d = nc.dram_tensor("msk", [P, P], BF16, kind="ExternalInput")
    # unnormalized PV output, (d, n)-major: rows 0:64 even head of pair,
    # 64:128 odd head; host transposes + normalizes.
    out_d = nc.dram_tensor("out", [NP, P, N], F32, kind="ExternalOutput")
    dsum_d = nc.dram_tensor("dsum", [NP, QC, 2, CW], F32,
                            kind="ExternalOutput")
    warm_d = nc.dram_tensor("warm", [P, 2 * P], BF16, kind="ExternalOutput")

    with tile.TileContext(nc) as tc:
        with (
            tc.tile_pool(name="big", bufs=1) as big,
            tc.tile_pool(name="ps", bufs=2, space="PSUM") as ps_pool,
            tc.tile_pool(name="prj", bufs=2, space="PSUM") as prj_pool,
            tc.tile_pool(name="po", bufs=1, space="PSUM") as po_pool,
            tc.tile_pool(name="pd", bufs=1, space="PSUM") as pd_pool,
            tc.tile_pool(name="sm", bufs=1) as sm,
        ):
            # ---- warm-up: PE matmul chain + exp table preload, during DMA
            wrm = big.tile([P, P], BF16, tag="wrm", name="wrm")
            nc.vector.memset(wrm[:, :], 0.0)
            wex = sm.tile([P, P], BF16, tag="wex", name="wex")
            nc.scalar.activation(wex[:, :], wrm[:, :], EXPF, scale=0.125)
            wps = prj_pool.tile([P, CW], F32, tag="prj", name="wps")
            for i in range(WARM_MM):
                nc.tensor.matmul(wps[:, 0:P], wrm[:, :], wrm[:, :],
                                 start=(i == 0), stop=(i == WARM_MM - 1))
            wout = sm.tile([P, P], BF16, tag="wout", name="wout")
            nc.vector.tensor_copy(wout[:, :], wps[:, 0:P])
            nc.sync.dma_start(warm_d[:, 0:P], wout[:, :])
            nc.sync.dma_start(warm_d[:, P:2 * P], wex[:, :])

            # doubled mask tile for single-instruction pair masking
            msk_sb = big.tile([P, 2 * P], BF16, tag="msk", name="msk_sb")
            nc.sync.dma_start(msk_sb[:, 0:P], msk_d[:, :])
            nc.sync.dma_start(msk_sb[:, P:2 * P], msk_d[:, :])
            ones_sb = big.tile([P, 1], BF16, tag="ones", name="ones_sb")
            nc.vector.memset(ones_sb[:, :], 1.0)

            # merged per-tensor tiles: one 3D-AP DMA per (tensor, col range)
            # keeps the Sync queue's per-DMA issue cost off the startup path.
            xtall = big.tile([P, FT * N], BF16, tag="xtall", name="xtall")
            wall = {wname: big.tile([P, FT * GC], BF16, tag=f"w{wname}",
                                    name=f"w{wname}")
                    for wname in ("q", "k", "v")}

            def xt(ft, a, b):
                return xtall[:, ft * N + a:ft * N + b]

            def wsl(wname, ft, a, b):
                return wall[wname][:, ft * GC + a:ft * GC + b]

            def wdst(wname, lo, hi):
                return wall[wname].rearrange(
                    "p (ft gc) -> p ft gc", ft=FT)[:, :, lo:hi]

            def wsrc(wd, lo, hi):
                return wd[:, :].rearrange(
                    "(ft p) gc -> p ft gc", p=P)[:, :, lo:hi]

            def xdst(lo, hi):
                return xtall.rearrange(
                    "p (ft n) -> p ft n", ft=FT)[:, :, lo:hi]

            def xsrc(lo, hi):
                return xT_d[:, :].rearrange(
                    "(ft p) n -> p ft n", p=P)[:, :, lo:hi]

            # need order: q/k jt0 slices, x col-chunk 0, wv, rest.
            nc.sync.dma_start(wdst("q", 0, P), wsrc(wq_d, 0, P))
            nc.sync.dma_start(wdst("k", 0, P), wsrc(wk_d, 0, P))
            nc.sync.dma_start(xdst(0, CW), xsrc(0, CW))
            nc.sync.dma_start(wdst("v", 0, GC), wsrc(wv_d, 0, GC))
            nc.sync.dma_start(xdst(CW, N), xsrc(CW, N))
            nc.sync.dma_start(wdst("q", P, GC), wsrc(wq_d, P, GC))
            nc.sync.dma_start(wdst("k", P, GC), wsrc(wk_d, P, GC))

            qt_sb = [big.tile([P, N], BF16, tag=f"qt{j}", name=f"qt{j}")
                     for j in range(JT)]
            kt_sb = [big.tile([P, N], BF16, tag=f"kt{j}", name=f"kt{j}")
                     for j in range(JT)]
            v_sb = [big.tile([P, GC], BF16, tag=f"v{s}", name=f"v{s}")
                    for s in range(ST)]

            def proj_qk_chunk(dst, wname, jt, c):
                # dst[jt][:, c*CW:+CW] = W[:, jt rows]^T @ xT[:, c chunk]
                pq = prj_pool.tile([P, CW], F32, tag="prj", name="pq")
                for ft in range(FT):
                    nc.tensor.matmul(
                        pq[:, 0:CW],
                        wsl(wname, ft, jt * P, (jt + 1) * P),
                        xt(ft, c * CW, (c + 1) * CW),
                        start=(ft == 0), stop=(ft == FT - 1))
                nc.vector.tensor_copy(dst[jt][:, c * CW:(c + 1) * CW],
                                      pq[:, 0:CW])

            def proj_v(st):
                # v_sb[st] = x rows [st] @ Wv   (partition = seq)
                pv = prj_pool.tile([P, CW], F32, tag="prj", name="pv")
                for ft in range(FT):
                    nc.tensor.matmul(pv[:, 0:GC],
                                     xt(ft, st * P, (st + 1) * P),
                                     wsl("v", ft, 0, GC),
                                     start=(ft == 0), stop=(ft == FT - 1))
                nc.vector.tensor_copy(v_sb[st][:, :], pv[:, 0:GC])

            def gen_qk(dst, wname, jt, c):
                pq = prj_pool.tile([P, CW], F32, tag="prj", name="pq")
                for ft in range(FT):
                    nc.tensor.matmul(
                        pq[:, 0:CW],
                        wsl(wname, ft, jt * P, (jt + 1) * P),
                        xt(ft, c * CW, (c + 1) * CW),
                        start=(ft == 0), stop=(ft == FT - 1))
                    yield
                nc.vector.tensor_copy(dst[jt][:, c * CW:(c + 1) * CW],
                                      pq[:, 0:CW])

            def gen_v(st):
                pv = prj_pool.tile([P, CW], F32, tag="prj", name="pv")
                for ft in range(FT):
                    nc.tensor.matmul(pv[:, 0:GC],
                                     xt(ft, st * P, (st + 1) * P),
                                     wsl("v", ft, 0, GC),
                                     start=(ft == 0), stop=(ft == FT - 1))
                    yield
                nc.vector.tensor_copy(v_sb[st][:, :], pv[:, 0:GC])

            def chain_q(jt, c):
                return lambda: gen_qk(qt_sb, "q", jt, c)

            def chain_k(jt, c):
                return lambda: gen_qk(kt_sb, "k", jt, c)

            def chain_v(st):
                return lambda: gen_v(st)

            # single-matmul background feeder: projection matmuls are fed
            # one at a time between attention pair-groups so the next
            # pair's LDWEIGHTS hides under a solo matmul's stream.
            feeder_state = {"pending": [], "active": None, "done": 0}

            def feeder_add(make_gen):
                feeder_state["pending"].append(make_gen)

            def feeder_step(n):
                for _ in range(n):
                    if feeder_state["active"] is None:
                        if not feeder_state["pending"]:
                            return
                        feeder_state["active"] = \
                            feeder_state["pending"].pop(0)()
                    try:
                        next(feeder_state["active"])
                    except StopIteration:
                        feeder_state["active"] = None
                    feeder_state["done"] += 1

            # background projection schedule per (pair, qc) chunk
            bg = {}
            bg[(0, 0)] = [chain_v(0), chain_v(1), chain_v(2), chain_v(3),
                          chain_q(0, 1), chain_k(0, 1)]
            bg[(0, 1)] = [chain_v(4), chain_v(5), chain_v(6), chain_v(7),
                          chain_q(0, 2), chain_k(0, 2)]
            bg[(0, 2)] = [chain_v(8), chain_v(9), chain_v(10), chain_v(11),
                          chain_q(0, 3), chain_k(0, 3)]
            bg[(0, 3)] = [chain_v(12), chain_v(13), chain_v(14),
                          chain_v(15), chain_q(1, 0), chain_k(1, 0)]
            for p in range(1, NP):
                for qc in range(QC):
                    if p == NP - 1 and qc == QC - 1:
                        bg[(p, qc)] = []
                    elif qc == QC - 1:
                        bg[(p, qc)] = [chain_q(p + 1, 0), chain_k(p + 1, 0)]
                    else:
                        bg[(p, qc)] = [chain_q(p, qc + 1),
                                       chain_k(p, qc + 1)]

            # upfront projections (needed before first attention iter)
            proj_qk_chunk(qt_sb, "q", 0, 0)
            proj_qk_chunk(kt_sb, "k", 0, 0)

            def attn_chunk(p, qc):
                jt = p
                c0, c1 = 2 * p * D, (2 * p + 1) * D
                nk = (qc + 1) * (CW // P)
                dn_n = 4 + qc          # 4 immediate emits + (nk-4)/4 grouped
                po = po_pool.tile([P, CW], F32, tag="po", name="po")
                pd = pd_pool.tile([33, CW], F32, tag="pd", name="pd")
                chains = bg[(p, qc)]
                for ch in chains:
                    feeder_add(ch)
                total = 9 * len(chains)
                base = [feeder_state["done"]]
                state = {"dn_i": 0, "pend": None, "cnt": 0}

                def pace(kj, slot):
                    want = min(total,
                               -(-total * (2 * kj + slot) // (2 * nk)))
                    need = base[0] + want - feeder_state["done"]
                    if need > 0:
                        feeder_step(need)

                def emit_denom(src, sl_):
                    nc.tensor.matmul(pd[0:1, sl_:CW], ones_sb[:, :],
                                     src[:, 0:CW - sl_],
                                     start=(state["dn_i"] == 0),
                                     stop=(state["dn_i"] == dn_n - 1))
                    nc.tensor.matmul(pd[32:33, sl_:CW], ones_sb[:, :],
                                     src[:, CW:2 * CW - sl_],
                                     start=(state["dn_i"] == 0),
                                     stop=(state["dn_i"] == dn_n - 1))
                    state["dn_i"] += 1

                def make_scores(kj):
                    sl = max(0, kj * P - qc * CW)
                    w = CW - sl
                    ps = ps_pool.tile([P, 2 * CW], F32, tag="ps", name="ps")
                    nc.tensor.matmul(
                        ps[:, 0:w],
                        kt_sb[jt][0:D, kj * P:(kj + 1) * P],
                        qt_sb[jt][0:D, qc * CW + sl:(qc + 1) * CW],
                        start=True, stop=True)
                    nc.tensor.matmul(
                        ps[:, CW:CW + w],
                        kt_sb[jt][D:P, kj * P:(kj + 1) * P],
                        qt_sb[jt][D:P, qc * CW + sl:(qc + 1) * CW],
                        start=True, stop=True)
                    ex = sm.tile([P, 2 * CW], BF16, tag="ex", name="ex",
                                 bufs=8)
                    if w == CW:
                        nc.scalar.activation(ex[:, :], ps[:, :],
                                             EXPF, scale=0.125)
                    else:
                        nc.scalar.activation(
                            ex.rearrange("p (two cw) -> p two cw",
                                         two=2)[:, :, 0:w],
                            ps.rearrange("p (two cw) -> p two cw",
                                         two=2)[:, :, 0:w],
                            EXPF, scale=0.125)
                    if kj * P >= qc * CW:  # diagonal: mask both heads' strip
                        nc.vector.tensor_mul(
                            ex.rearrange("p (two cw) -> p two cw",
                                         two=2)[:, :, 0:P],
                            ex.rearrange("p (two cw) -> p two cw",
                                         two=2)[:, :, 0:P],
                            msk_sb.rearrange("p (two w) -> p two w",
                                             two=2)[:, :, :])
                    return (ex, sl, w, kj)

                def emit_pv_denom(item):
                    ex, sl, w, kj = item
                    st_, sp_ = (kj == 0), (kj == nk - 1)
                    nc.tensor.matmul(po[0:D, sl:CW],
                                     v_sb[kj][:, c0:c0 + D],
                                     ex[:, 0:w], start=st_, stop=sp_)
                    nc.tensor.matmul(po[D:P, sl:CW],
                                     v_sb[kj][:, c1:c1 + D],
                                     ex[:, CW:CW + w], start=st_, stop=sp_)
                    if w == CW and kj * P < qc * CW:
                        # full-width off-diagonal: chain-sum DSUM_S deep
                        if state["pend"] is None:
                            state["pend"] = ex
                            state["cnt"] = 1
                        else:
                            ex2 = sm.tile([P, 2 * CW], BF16, tag="exs",
                                          name="exs", bufs=3)
                            nc.vector.tensor_add(ex2[:, :],
                                                 state["pend"][:, :],
                                                 ex[:, :])
                            state["pend"] = ex2
                            state["cnt"] += 1
                        if state["cnt"] == DSUM_S:
                            emit_denom(state["pend"], 0)
                            state["pend"] = None
                            state["cnt"] = 0
                    else:
                        emit_denom(ex, sl)

                prev = None
                for kj in range(nk):
                    cur = make_scores(kj)
                    pace(kj, 1)
                    if prev is not None:
                        emit_pv_denom(prev)
                    pace(kj, 2)
                    prev = cur
                emit_pv_denom(prev)
                if state["pend"] is not None:
                    emit_denom(state["pend"], 0)
                    state["pend"] = None
                # finalize: copy PV accumulator, plain (d, n)-major store
                ot = sm.tile([P, CW], F32, tag="ot", name="ot", bufs=2)
                nc.vector.tensor_copy(ot[:, :], po[:, :])
                nc.sync.dma_start(out_d[p, :, qc * CW:(qc + 1) * CW],
                                  ot[:, :])
                dn = sm.tile([33, CW], F32, tag="dn", name="dn", bufs=2)
                nc.vector.tensor_copy(dn[:, :], pd[:, :])
                nc.sync.dma_start(dsum_d[p, qc, 0], dn[0:1, :])
                nc.sync.dma_start(dsum_d[p, qc, 1], dn[32:33, :])

            for p in range(NP):
                for qc in range(QC):
                    attn_chunk(p, qc)
    print(f"graph built in {time.time()-t0:.1f}s; compiling...", flush=True)
    nc.compile()
    print(f"compiled at {time.time()-t0:.1f}s", flush=True)
    return nc


def _get_nc():
    global _NC_CACHE
    if _NC_CACHE is None:
        _NC_CACHE = _build()
    return _NC_CACHE


def make_in_maps(x, Wq, Wk, Wv):
    bf = ml_dtypes.bfloat16
    msk = np.triu(np.ones((P, P), dtype=np.float32)).astype(bf)
    in_maps = []
    for c in range(NCORES):
        b, g = c // 2, c % 2
        cols = slice(g * GC, (g + 1) * GC)
        in_maps.append({
            "xT": np.ascontiguousarray(np.asarray(x)[b].T).astype(bf),
            "wq": np.asarray(Wq)[:, cols].astype(bf),
            "wk": np.asarray(Wk)[:, cols].astype(bf),
            "wv": np.asarray(Wv)[:, cols].astype(bf),
            "msk": msk,
        })
    return in_maps


def gather_out(res):
    out = np.empty((B, N, F), dtype=np.float32)
    for c in range(NCORES):
        b, g = c // 2, c % 2
        o = res.results[c]["out"]                      # (NP, 128, N) f32
        ds = res.results[c]["dsum"]                    # (NP, QC, 2, CW)
        den = ds.transpose(0, 2, 1, 3).reshape(NP, 2, 1, N)
        o = o.reshape(NP, 2, D, N) / den               # normalize
        o = o.transpose(3, 0, 1, 2).reshape(N, GC)     # (n, h*d)
        out[b, :, g * GC:(g + 1) * GC] = o
    return out


def kernel(x, Wq, bq, Wk, bk, Wv, bv):
    in_maps = make_in_maps(x, Wq, Wk, Wv)
    res = run_bass_kernel_spmd(_get_nc(), in_maps, core_ids=list(range(NCORES)))
    return gather_out(res)


# revision 20
# speedup vs baseline: 1.0505x; 1.0505x over previous
"""Causal MHA (B=4, N=2048, F=1024, H=16, D=64) on 8 TRN2 NeuronCores.

Sharding: core c -> batch c//2, head-group c%2 (8 heads each). No
cross-core communication.

v4 = v2 head-pair packing + pipeline shift + projection interleave.
 - v2 structure kept: scores for head pair (2p, 2p+1) issue as adjacent
   row-group-packed matmuls into ONE [128, 2*CW] PSUM tile; one exp per
   kj covers both heads; PV packs the pair into po col groups.
 - PV/denom for kj are issued one iteration LATE (after scores/exp of
   kj+1), so the PE FIFO queue always has exp-independent work and
   never stalls on the scalar engine.
 - projection 8-matmul chains interleave between attention iterations
   (paced schedule), instead of running as serial blocks.
 - output stored (d, n)-major f32 straight from the PV accumulator (one
   copy + one plain DMA per chunk; NO DMA transposes); host fixes
   layout + normalizes.
 - denominators: full off-diagonal ex tiles chain-summed 4-deep on DVE
   before each ones-matmul (half the denominator matmuls of v2).
 - diagonal causal masks: both heads' 128-col strips in ONE strided
   tensor_mul against a doubled mask tile.
 - PE warm-up matmul chain + dummy exp (ACT table preload) overlap the
   initial DMAs; input DMAs are chunked so the first scores issue ~5us
   in (w row-blocks jt0 first, x column-chunk 0, wv, rest).
"""

import sys
import time

sys.path.insert(0, "/opt/trn_rl_repo")

import ml_dtypes
import numpy as np

import concourse.bacc as bacc
import concourse.mybir as mybir
import concourse.tile as tile
from concourse.bass_utils import run_bass_kernel_spmd

B, N, F, H = 4, 2048, 1024, 16
D = 64
NCORES = 8
HL = H // 2          # heads per core
NP = HL // 2         # head pairs per core (4)
GC = HL * D          # per-core projection width (512)
P = 128
FT = F // P          # 8 contraction tiles
JT = GC // P         # 4 row tiles of QT/KT (one per head pair)
ST = N // P          # 16 seq tiles
CW = 512             # query chunk width
QC = N // CW         # 4 query chunks
BF16 = mybir.dt.bfloat16
F32 = mybir.dt.float32
EXPF = mybir.ActivationFunctionType.Exp
WARM_MM = 96         # warm-up matmuls (~10us: bridge the input-DMA window)
DSUM_S = 8           # ex tiles chain-summed per denominator matmul

_NC_CACHE = None


def _build():
    t0 = time.time()
    print("building bass graph...", flush=True)
    nc = bacc.Bacc("TRN2", target_bir_lowering=False, debug=False,
                   num_devices=NCORES)
    xT_d = nc.dram_tensor("xT", [F, N], BF16, kind="ExternalInput")
    wq_d = nc.dram_tensor("wq", [F, GC], BF16, kind="ExternalInput")
    wk_d = nc.dram_tensor("wk", [F, GC], BF16, kind="ExternalInput")
    wv_d = nc.dram_tensor("wv", [F, GC], BF16, kind="ExternalInput")
    msk_d = nc.dram_tensor("msk", [P, P], BF16, kind="ExternalInput")
    # unnormalized PV output, (d, n)-major: rows 0:64 even head of pair,
    # 64:128 odd head; host transposes + normalizes.
    out_d = nc.dram_tensor("out", [NP, P, N], F32, kind="ExternalOutput")
    dsum_d = nc.dram_tensor("dsum", [NP, QC, 2, CW], F32,
                            kind="ExternalOutput")
    warm_d = nc.dram_tensor("warm", [P, 2 * P], BF16, kind="ExternalOutput")

    with tile.TileContext(nc) as tc:
        with (
            tc.tile_pool(name="big", bufs=1) as big,
            tc.tile_pool(name="ps", bufs=2, space="PSUM") as ps_pool,
            tc.tile_pool(name="prj", bufs=2, space="PSUM") as prj_pool,
            tc.tile_pool(name="po", bufs=1, space="PSUM") as po_pool,
            tc.tile_pool(name="pd", bufs=1, space="PSUM") as pd_pool,
            tc.tile_pool(name="sm", bufs=1) as sm,
        ):
            # ---- warm-up: PE matmul chain + exp table preload, during DMA
            wrm = big.tile([P, P], BF16, tag="wrm", name="wrm")
            nc.vector.memset(wrm[:, :], 0.0)
            wex = sm.tile([P, P], BF16, tag="wex", name="wex")
            nc.scalar.activation(wex[:, :], wrm[:, :], EXPF, scale=0.125)
            wps = prj_pool.tile([P, CW], F32, tag="prj", name="wps")
            for i in range(WARM_MM):
                nc.tensor.matmul(wps[:, 0:P], wrm[:, :], wrm[:, :],
                                 start=(i == 0), stop=(i == WARM_MM - 1))
            wout = sm.tile([P, P], BF16, tag="wout", name="wout")
            nc.vector.tensor_copy(wout[:, :], wps[:, 0:P])
            nc.sync.dma_start(warm_d[:, 0:P], wout[:, :])
            nc.sync.dma_start(warm_d[:, P:2 * P], wex[:, :])

            # doubled mask tile for single-instruction pair masking
            msk_sb = big.tile([P, 2 * P], BF16, tag="msk", name="msk_sb")
            nc.sync.dma_start(msk_sb[:, 0:P], msk_d[:, :])
            nc.sync.dma_start(msk_sb[:, P:2 * P], msk_d[:, :])
            ones_sb = big.tile([P, 1], BF16, tag="ones", name="ones_sb")
            nc.vector.memset(ones_sb[:, :], 1.0)

            # merged per-tensor tiles: one 3D-AP DMA per (tensor, col range)
            # keeps the Sync queue's per-DMA issue cost off the startup path.
            xtall = big.tile([P, FT * N], BF16, tag="xtall", name="xtall")
            wall = {wname: big.tile([P, FT * GC], BF16, tag=f"w{wname}",
                                    name=f"w{wname}")
                    for wname in ("q", "k", "v")}

            def xt(ft, a, b):
                return xtall[:, ft * N + a:ft * N + b]

            def wsl(wname, ft, a, b):
                return wall[wname][:, ft * GC + a:ft * GC + b]

            def wdst(wname, lo, hi):
                return wall[wname].rearrange(
                    "p (ft gc) -> p ft gc", ft=FT)[:, :, lo:hi]

            def wsrc(wd, lo, hi):
                return wd[:, :].rearrange(
                    "(ft p) gc -> p ft gc", p=P)[:, :, lo:hi]

            def xdst(lo, hi):
                return xtall.rearrange(
                    "p (ft n) -> p ft n", ft=FT)[:, :, lo:hi]

            def xsrc(lo, hi):
                return xT_d[:, :].rearrange(
                    "(ft p) n -> p ft n", p=P)[:, :, lo:hi]

            # need order: q/k jt0 slices, x col-chunk 0, wv, rest.
            nc.sync.dma_start(wdst("q", 0, P), wsrc(wq_d, 0, P))
            nc.sync.dma_start(wdst("k", 0, P), wsrc(wk_d, 0, P))
            nc.sync.dma_start(xdst(0, CW), xsrc(0, CW))
            nc.sync.dma_start(wdst("v", 0, GC), wsrc(wv_d, 0, GC))
            nc.sync.dma_start(xdst(CW, N), xsrc(CW, N))
            nc.sync.dma_start(wdst("q", P, GC), wsrc(wq_d, P, GC))
            nc.sync.dma_start(wdst("k", P, GC), wsrc(wk_d, P, GC))

            qt_sb = [big.tile([P, N], BF16, tag=f"qt{j}", name=f"qt{j}")
                     for j in range(JT)]
            kt_sb = [big.tile([P, N], BF16, tag=f"kt{j}", name=f"kt{j}")
                     for j in range(JT)]
            v_sb = [big.tile([P, GC], BF16, tag=f"v{s}", name=f"v{s}")
                    for s in range(ST)]

            def proj_qk_chunk(dst, wname, jt, c):
                # dst[jt][:, c*CW:+CW] = W[:, jt rows]^T @ xT[:, c chunk]
                pq = prj_pool.tile([P, CW], F32, tag="prj", name="pq")
                for ft in range(FT):
                    nc.tensor.matmul(
                        pq[:, 0:CW],
                        wsl(wname, ft, jt * P, (jt + 1) * P),
                        xt(ft, c * CW, (c + 1) * CW),
                        start=(ft == 0), stop=(ft == FT - 1))
                nc.vector.tensor_copy(dst[jt][:, c * CW:(c + 1) * CW],
                                      pq[:, 0:CW])

            def proj_v(st):
                # v_sb[st] = x rows [st] @ Wv   (partition = seq)
                pv = prj_pool.tile([P, CW], F32, tag="prj", name="pv")
                for ft in range(FT):
                    nc.tensor.matmul(pv[:, 0:GC],
                                     xt(ft, st * P, (st + 1) * P),
                                     wsl("v", ft, 0, GC),
                                     start=(ft == 0), stop=(ft == FT - 1))
                nc.vector.tensor_copy(v_sb[st][:, :], pv[:, 0:GC])

            def chain_q(jt, c):
                return lambda: proj_qk_chunk(qt_sb, "q", jt, c)

            def chain_k(jt, c):
                return lambda: proj_qk_chunk(kt_sb, "k", jt, c)

            def chain_v(st):
                return lambda: proj_v(st)

            # background projection schedule per (pair, qc) chunk
            bg = {}
            bg[(0, 0)] = [chain_v(0), chain_v(1), chain_v(2), chain_v(3),
                          chain_q(0, 1), chain_k(0, 1)]
            bg[(0, 1)] = [chain_v(4), chain_v(5), chain_v(6), chain_v(7),
                          chain_q(0, 2), chain_k(0, 2)]
            bg[(0, 2)] = [chain_v(8), chain_v(9), chain_v(10), chain_v(11),
                          chain_q(0, 3), chain_k(0, 3)]
            bg[(0, 3)] = [chain_v(12), chain_v(13), chain_v(14),
                          chain_v(15), chain_q(1, 0), chain_k(1, 0)]
            for p in range(1, NP):
                for qc in range(QC):
                    if p == NP - 1 and qc == QC - 1:
                        bg[(p, qc)] = []
                    elif qc == QC - 1:
                        bg[(p, qc)] = [chain_q(p + 1, 0), chain_k(p + 1, 0)]
                    else:
                        bg[(p, qc)] = [chain_q(p, qc + 1),
                                       chain_k(p, qc + 1)]

            # upfront projections (needed before first attention iter)
            proj_qk_chunk(qt_sb, "q", 0, 0)
            proj_qk_chunk(kt_sb, "k", 0, 0)

            def attn_chunk(p, qc):
                jt = p
                c0, c1 = 2 * p * D, (2 * p + 1) * D
                nk = (qc + 1) * (CW // P)
                # 4 immediate emits + ceil((nk-4)/DSUM_S) grouped emits
                dn_n = 4 + max(0, -(-(nk - 4) // DSUM_S))
                po = po_pool.tile([P, CW], F32, tag="po", name="po")
                pd = pd_pool.tile([33, CW], F32, tag="pd", name="pd")
                chains = bg[(p, qc)]
                issued = [0]
                state = {"dn_i": 0, "pend": None, "cnt": 0}

                def pace(kj):
                    want = min(len(chains),
                               -(-len(chains) * (kj + 1) // nk))
                    while issued[0] < want:
                        chains[issued[0]]()
                        issued[0] += 1

                def emit_denom(src, sl_):
                    nc.tensor.matmul(pd[0:1, sl_:CW], ones_sb[:, :],
                                     src[:, 0:CW - sl_],
                                     start=(state["dn_i"] == 0),
                                     stop=(state["dn_i"] == dn_n - 1))
                    nc.tensor.matmul(pd[32:33, sl_:CW], ones_sb[:, :],
                                     src[:, CW:2 * CW - sl_],
                                     start=(state["dn_i"] == 0),
                                     stop=(state["dn_i"] == dn_n - 1))
                    state["dn_i"] += 1

                def make_scores(kj):
                    sl = max(0, kj * P - qc * CW)
                    w = CW - sl
                    ps = ps_pool.tile([P, 2 * CW], F32, tag="ps", name="ps")
                    nc.tensor.matmul(
                        ps[:, 0:w],
                        kt_sb[jt][0:D, kj * P:(kj + 1) * P],
                        qt_sb[jt][0:D, qc * CW + sl:(qc + 1) * CW],
                        start=True, stop=True)
                    nc.tensor.matmul(
                        ps[:, CW:CW + w],
                        kt_sb[jt][D:P, kj * P:(kj + 1) * P],
                        qt_sb[jt][D:P, qc * CW + sl:(qc + 1) * CW],
                        start=True, stop=True)
                    ex = sm.tile([P, 2 * CW], BF16, tag="ex", name="ex",
                                 bufs=8)
                    if w == CW:
                        nc.scalar.activation(ex[:, :], ps[:, :],
                                             EXPF, scale=0.125)
                    else:
                        nc.scalar.activation(
                            ex.rearrange("p (two cw) -> p two cw",
                                         two=2)[:, :, 0:w],
                            ps.rearrange("p (two cw) -> p two cw",
                                         two=2)[:, :, 0:w],
                            EXPF, scale=0.125)
                    if kj * P >= qc * CW:  # diagonal: mask both heads' strip
                        nc.vector.tensor_mul(
                            ex.rearrange("p (two cw) -> p two cw",
                                         two=2)[:, :, 0:P],
                            ex.rearrange("p (two cw) -> p two cw",
                                         two=2)[:, :, 0:P],
                            msk_sb.rearrange("p (two w) -> p two w",
                                             two=2)[:, :, :])
                    return (ex, sl, w, kj)

                def emit_pv_denom(item):
                    ex, sl, w, kj = item
                    st_, sp_ = (kj == 0), (kj == nk - 1)
                    nc.tensor.matmul(po[0:D, sl:CW],
                                     v_sb[kj][:, c0:c0 + D],
                                     ex[:, 0:w], start=st_, stop=sp_)
                    nc.tensor.matmul(po[D:P, sl:CW],
                                     v_sb[kj][:, c1:c1 + D],
                                     ex[:, CW:CW + w], start=st_, stop=sp_)
                    if w == CW and kj * P < qc * CW:
                        # full-width off-diagonal: chain-sum DSUM_S deep
                        if state["pend"] is None:
                            state["pend"] = ex
                            state["cnt"] = 1
                        else:
                            ex2 = sm.tile([P, 2 * CW], BF16, tag="exs",
                                          name="exs", bufs=3)
                            nc.vector.tensor_add(ex2[:, :],
                                                 state["pend"][:, :],
                                                 ex[:, :])
                            state["pend"] = ex2
                            state["cnt"] += 1
                        if state["cnt"] == DSUM_S:
                            emit_denom(state["pend"], 0)
                            state["pend"] = None
                            state["cnt"] = 0
                    else:
                        emit_denom(ex, sl)

                prev = None
                for kj in range(nk):
                    cur = make_scores(kj)
                    pace(kj)
                    if prev is not None:
                        emit_pv_denom(prev)
                    prev = cur
                emit_pv_denom(prev)
                if state["pend"] is not None:
                    emit_denom(state["pend"], 0)
                    state["pend"] = None
                # finalize: copy PV accumulator, plain (d, n)-major store
                ot = sm.tile([P, CW], F32, tag="ot", name="ot", bufs=2)
                nc.vector.tensor_copy(ot[:, :], po[:, :])
                nc.sync.dma_start(out_d[p, :, qc * CW:(qc + 1) * CW],
                                  ot[:, :])
                dn = sm.tile([33, CW], F32, tag="dn", name="dn", bufs=2)
                nc.vector.tensor_copy(dn[:, :], pd[:, :])
                nc.sync.dma_start(dsum_d[p, qc, 0], dn[0:1, :])
                nc.sync.dma_start(dsum_d[p, qc, 1], dn[32:33, :])

            for p in range(NP):
                for qc in range(QC):
                    attn_chunk(p, qc)
    print(f"graph built in {time.time()-t0:.1f}s; compiling...", flush=True)
    nc.compile()
    print(f"compiled at {time.time()-t0:.1f}s", flush=True)
    return nc


def _get_nc():
    global _NC_CACHE
    if _NC_CACHE is None:
        _NC_CACHE = _build()
    return _NC_CACHE


def make_in_maps(x, Wq, Wk, Wv):
    bf = ml_dtypes.bfloat16
    msk = np.triu(np.ones((P, P), dtype=np.float32)).astype(bf)
    in_maps = []
    for c in range(NCORES):
        b, g = c // 2, c % 2
        cols = slice(g * GC, (g + 1) * GC)
        in_maps.append({
            "xT": np.ascontiguousarray(np.asarray(x)[b].T).astype(bf),
            "wq": np.asarray(Wq)[:, cols].astype(bf),
            "wk": np.asarray(Wk)[:, cols].astype(bf),
            "wv": np.asarray(Wv)[:, cols].astype(bf),
            "msk": msk,
        })
    return in_maps


def gather_out(res):
    out = np.empty((B, N, F), dtype=np.float32)
    for c in range(NCORES):
        b, g = c // 2, c % 2
        o = res.results[c]["out"]                      # (NP, 128, N) f32
        ds = res.results[c]["dsum"]                    # (NP, QC, 2, CW)
        den = ds.transpose(0, 2, 1, 3).reshape(NP, 2, 1, N)
        o = o.reshape(NP, 2, D, N) / den               # normalize
        o = o.transpose(3, 0, 1, 2).reshape(N, GC)     # (n, h*d)
        out[b, :, g * GC:(g + 1) * GC] = o
    return out


def kernel(x, Wq, bq, Wk, bk, Wv, bv):
    in_maps = make_in_maps(x, Wq, Wk, Wv)
    res = run_bass_kernel_spmd(_get_nc(), in_maps, core_ids=list(range(NCORES)))
    return gather_out(res)
